# revision 1
# baseline (speedup 1.0000x reference)
"""KANLinear (B-spline) Trainium2 kernel.

Math: out = silu(x) @ Wb^T + einsum('nik,oik->no', Bspline(x), Ws*scaler)
Grid is uniform: knots at t = j (j=0..11) where t = x/1.2 + 5.5.
Closed form per element: m = floor(t) in [0,10], u = t - m,
  b_k = mask[m==k]*P0(u) + mask[m==k+1]*P1(u) + mask[m==k+2]*P2(u) + mask[m==k+3]*P3(u)
  P0 = u^3/6, P1 = (-3u^3+3u^2+3u+1)/6, P2 = (3u^3-6u^2+4)/6, P3 = (1-u)^3/6
Sharding: data-parallel over N across 8 cores; weights replicated.
GEMM: K = 512 (silu base) + 512*8 (spline) = 4608, bf16 inputs, f32 PSUM.
"""
import sys, os
sys.path.insert(0, '/opt/trn_rl_repo')
import numpy as np
import ml_dtypes
from contextlib import ExitStack

import concourse.bass as bass
import concourse.bacc as bacc
import concourse.tile as tile
import concourse.mybir as mybir
from concourse.bass_utils import run_bass_kernel_spmd

f32 = mybir.dt.float32
bf16 = mybir.dt.bfloat16
Alu = mybir.AluOpType
Act = mybir.ActivationFunctionType

N_TOTAL, IN_F, OUT_F = 32768, 512, 512
NCORES = 8
N_CORE = N_TOTAL // NCORES          # 4096
NBLK = 512                          # rows per block
NBLOCKS = N_CORE // NBLK            # 8
KT = 4 + 8 * 4                      # 36 K-tiles of 128: 4 silu + 8 coef * 4 i-tiles
INV_H = 1.0 / 1.2
T_OFF = 5.5

_cache = {}


def _build():
    if 'nc' in _cache:
        return _cache['nc']
    nc = bacc.Bacc("TRN2", target_bir_lowering=False, debug=False, num_devices=NCORES)
    for cv in (T_OFF, INV_H, -6.6, 1.2):
        th = nc.alloc_sbuf_tensor(f"constk-{cv}", [128, 1], f32)
        nc.gpsimd.memset(th.ap(), cv)
        nc.const_aps.aps[(f32, cv)] = th.ap()
    nc.all_engine_barrier()
    x_d = nc.dram_tensor("x", [N_CORE, IN_F], f32, kind="ExternalInput").ap()
    w_d = nc.dram_tensor("w", [KT * 128, OUT_F], bf16, kind="ExternalInput").ap()
    id_d = nc.dram_tensor("ident", [128, 128], f32, kind="ExternalInput").ap()
    y_d = nc.dram_tensor("y", [N_CORE, OUT_F], f32, kind="ExternalOutput").ap()

    with tile.TileContext(nc) as tc, ExitStack() as ctx:
        wpool = ctx.enter_context(tc.tile_pool(name="w", bufs=1))
        xpool = ctx.enter_context(tc.tile_pool(name="x", bufs=3))
        npool = ctx.enter_context(tc.tile_pool(name="tnat", bufs=8))
        tpool = ctx.enter_context(tc.tile_pool(name="tT", bufs=2))
        kpool = ctx.enter_context(tc.tile_pool(name="ktiles", bufs=1))
        tmp = ctx.enter_context(tc.tile_pool(name="tmp", bufs=1))
        pt_pool = ctx.enter_context(tc.tile_pool(name="ptrans", bufs=2, space="PSUM"))
        po_pool = ctx.enter_context(tc.tile_pool(name="pout", bufs=4, space="PSUM"))

        w_s = wpool.tile([128, KT * OUT_F], bf16, tag="w")
        ident = wpool.tile([128, 128], f32, tag="ident")
        nc.sync.dma_start(ident[:], id_d[:])
        for kt in range(KT):
            nc.sync.dma_start(w_s[:, kt * OUT_F:(kt + 1) * OUT_F],
                              w_d[kt * 128:(kt + 1) * 128, :])

        for blk in range(NBLOCKS):
            r0 = blk * NBLK
            # load x block and compute t = relu(x/1.2 + 5.5) in natural layout
            tnat = []
            for nt in range(4):
                xt = xpool.tile([128, IN_F], f32, tag="xin")
                nc.sync.dma_start(xt[:], x_d[r0 + nt * 128: r0 + (nt + 1) * 128, :])
                tn = npool.tile([128, IN_F], f32, tag="tnat")
                nc.scalar.activation(tn[:], xt[:], Act.Relu, bias=T_OFF, scale=INV_H)
                tnat.append(tn)
            # K-tile buffer for this block: [128, KT*NBLK] bf16
            kbuf = kpool.tile([128, KT * NBLK], bf16, tag="kbuf")

            for it in range(4):
                # transpose t[:, it*128:+128] from all 4 n-tiles -> tT [128i, 512n]
                ptr = pt_pool.tile([128, NBLK], f32, tag="ptr")
                for nt in range(4):
                    nc.tensor.transpose(ptr[:, nt * 128:(nt + 1) * 128],
                                        tnat[nt][:, it * 128:(it + 1) * 128], ident[:])
                tT = tpool.tile([128, NBLK], f32, tag="tT")
                nc.scalar.copy(tT[:], ptr[:])

                # silu K-tile: silu(x) = silu(1.2*t - 6.6)
                nc.scalar.activation(kbuf[:, it * NBLK:(it + 1) * NBLK], tT[:],
                                     Act.Silu, bias=-6.6, scale=1.2)

                # clamped t, u, m
                tcl = tmp.tile([128, NBLK], f32, tag="tcl")
                nc.vector.tensor_scalar(tcl[:], tT[:], 10.9999, None, Alu.min)
                # steps g_j = [t >= j], m = sum(g_j), u = t - m  (no mod/floor in ISA)
                g = []
                for j in range(1, 11):
                    gj = tmp.tile([128, NBLK], f32, tag=f"g{j}")
                    nc.vector.tensor_scalar(gj[:], tcl[:], float(j), None, Alu.is_ge)
                    g.append(gj)
                macc = tmp.tile([128, NBLK], f32, tag="macc0")
                nc.vector.tensor_add(macc[:], g[0][:], g[1][:])
                for j in range(2, 10):
                    nmacc = tmp.tile([128, NBLK], f32, tag=f"macc{(j-1) % 2}")
                    nc.vector.tensor_add(nmacc[:], macc[:], g[j][:])
                    macc = nmacc
                u = tmp.tile([128, NBLK], f32, tag="u")
                nc.vector.tensor_sub(u[:], tcl[:], macc[:])
                u2 = tmp.tile([128, NBLK], f32, tag="u2")
                nc.vector.tensor_mul(u2[:], u[:], u[:])
                u3 = tmp.tile([128, NBLK], f32, tag="u3")
                nc.vector.tensor_mul(u3[:], u2[:], u[:])

                # cubic pieces
                P0 = tmp.tile([128, NBLK], f32, tag="P0")
                nc.vector.tensor_scalar(P0[:], u3[:], 1.0 / 6.0, None, Alu.mult)
                s_ = tmp.tile([128, NBLK], f32, tag="s_")
                nc.vector.tensor_add(s_[:], u[:], u2[:])
                q1 = tmp.tile([128, NBLK], f32, tag="q1")
                nc.vector.tensor_scalar(q1[:], s_[:], 0.5, 1.0 / 6.0, Alu.mult, Alu.add)
                P1 = tmp.tile([128, NBLK], f32, tag="P1")
                nc.vector.scalar_tensor_tensor(P1[:], u3[:], -0.5, q1[:], Alu.mult, Alu.add)
                q2 = tmp.tile([128, NBLK], f32, tag="q2")
                nc.vector.tensor_scalar(q2[:], u2[:], -1.0, 2.0 / 3.0, Alu.mult, Alu.add)
                P2 = tmp.tile([128, NBLK], f32, tag="P2")
                nc.vector.scalar_tensor_tensor(P2[:], u3[:], 0.5, q2[:], Alu.mult, Alu.add)
                dq = tmp.tile([128, NBLK], f32, tag="dq")
                nc.vector.tensor_sub(dq[:], u2[:], u[:])
                q3 = tmp.tile([128, NBLK], f32, tag="q3")
                nc.vector.tensor_scalar(q3[:], dq[:], 0.5, 1.0 / 6.0, Alu.mult, Alu.add)
                P3 = tmp.tile([128, NBLK], f32, tag="P3")
                nc.vector.scalar_tensor_tensor(P3[:], u3[:], -1.0 / 6.0, q3[:], Alu.mult, Alu.add)

                # interval masks: mask_j = g_j - g_{j+1}; ends from step complements
                masks = []
                m0 = tmp.tile([128, NBLK], f32, tag="m0")
                nc.gpsimd.tensor_scalar(m0[:], g[0][:], -1.0, 1.0, Alu.mult, Alu.add)
                masks.append(m0)
                for j in range(1, 10):
                    mj = tmp.tile([128, NBLK], f32, tag=f"m{j}")
                    nc.gpsimd.tensor_sub(mj[:], g[j - 1][:], g[j][:])
                    masks.append(mj)
                masks.append(g[9])

                # combine: b_k -> kbuf tile (4 + k*4 + it)
                for k in range(8):
                    t1 = tmp.tile([128, NBLK], f32, tag="t1")
                    nc.vector.tensor_mul(t1[:], masks[k][:], P0[:])
                    t2 = tmp.tile([128, NBLK], f32, tag="t2")
                    nc.vector.tensor_mul(t2[:], masks[k + 1][:], P1[:])
                    t12 = tmp.tile([128, NBLK], f32, tag="t12")
                    nc.vector.tensor_add(t12[:], t1[:], t2[:])
                    t3 = tmp.tile([128, NBLK], f32, tag="t3")
                    nc.gpsimd.tensor_mul(t3[:], masks[k + 2][:], P2[:])
                    t4 = tmp.tile([128, NBLK], f32, tag="t4")
                    nc.gpsimd.tensor_mul(t4[:], masks[k + 3][:], P3[:])
                    t34 = tmp.tile([128, NBLK], f32, tag="t34")
                    nc.vector.tensor_add(t34[:], t3[:], t4[:])
                    kslot = 4 + k * 4 + it
                    nc.vector.tensor_add(kbuf[:, kslot * NBLK:(kslot + 1) * NBLK],
                                         t12[:], t34[:])

            # GEMM: for each n-sub row tile accumulate over all K tiles
            for nsub in range(4):
                po = po_pool.tile([128, OUT_F], f32, tag="po")
                for kt in range(KT):
                    nc.tensor.matmul(
                        po[:],
                        kbuf[:, kt * NBLK + nsub * 128: kt * NBLK + (nsub + 1) * 128],
                        w_s[:, kt * OUT_F:(kt + 1) * OUT_F],
                        start=(kt == 0), stop=(kt == KT - 1))
                yo = xpool.tile([128, OUT_F], f32, tag="yout")
                nc.scalar.copy(yo[:], po[:])
                nc.sync.dma_start(y_d[r0 + nsub * 128: r0 + (nsub + 1) * 128, :], yo[:])

    nc.compile()
    _cache['nc'] = nc
    return nc


def _prep_w(base_weight, spline_weight, spline_scaler):
    sw = spline_weight * spline_scaler[..., None]        # [out, in, 8]
    w = np.zeros((KT * 128, OUT_F), dtype=np.float32)
    w[0:512, :] = base_weight.T                          # silu branch
    for k in range(8):
        for it in range(4):
            kslot = 4 + k * 4 + it
            w[kslot * 128:(kslot + 1) * 128, :] = sw[:, it * 128:(it + 1) * 128, k].T
    return w.astype(ml_dtypes.bfloat16)


def kernel(x, base_weight, spline_weight, spline_scaler, grid):
    x = np.asarray(x, dtype=np.float32)
    w = _prep_w(np.asarray(base_weight, np.float32),
                np.asarray(spline_weight, np.float32),
                np.asarray(spline_scaler, np.float32))
    ident = np.eye(128, dtype=np.float32)
    nc = _build()
    in_maps = []
    for c in range(NCORES):
        in_maps.append({"x": np.ascontiguousarray(x[c * N_CORE:(c + 1) * N_CORE]),
                        "w": w, "ident": ident})
    res = run_bass_kernel_spmd(nc, in_maps, core_ids=list(range(NCORES)))
    out = np.concatenate([res.results[c]["y"] for c in range(NCORES)], axis=0)
    return out.astype(np.float32)



# revision 2
# speedup vs baseline: 22403.8319x; 22403.8319x over previous
"""KANLinear (B-spline) Trainium2 kernel — D-channel, K-outer pipelined GEMM.

See kernel_v2.py docstring for the math. v4: K-outer pipelined GEMM in
production order; x shipped as fp16 (host cast); per-i-tile kbuf ring (8)
so producers never wait on a whole block's GEMM; D9 written by the Pool
mul directly.
"""
import sys, os
sys.path.insert(0, '/opt/trn_rl_repo')
import numpy as np
from contextlib import ExitStack

import concourse.bass as bass
import concourse.bacc as bacc
import concourse.tile as tile
import concourse.mybir as mybir
from concourse.bass_utils import run_bass_kernel_spmd

f32 = mybir.dt.float32
f16 = mybir.dt.float16
Alu = mybir.AluOpType
Act = mybir.ActivationFunctionType

N_TOTAL, IN_F, OUT_F = 32768, 512, 512
NCORES = 8
N_CORE = N_TOTAL // NCORES          # 4096
NBLK = 512                          # rows per block
NBLOCKS = N_CORE // NBLK            # 8
NCH = 10                            # 1 silu + 9 spline D-channels
# channel 9 (active only for x>4.2, ~1e-5 of elements) is dropped: its
# post-cumsum weight is tiny; measured rel-err impact +6e-6.
KT = NCH * 4                        # 44 K-tiles of 128
BETA = [6.6 - 1.2 * c for c in range(11)]
# K-slot consumption order == production order: it-major, group-minor
SLOT_ORDER = [g * 4 + it for it in range(4) for g in range(NCH)]

_cache = {}


def _build():
    if 'nc' in _cache:
        return _cache['nc']
    nc = bacc.Bacc("TRN2", target_bir_lowering=False, debug=False, num_devices=NCORES)
    for cv in BETA[:10]:
        th = nc.alloc_sbuf_tensor(f"constk-{cv}", [128, 1], f32)
        nc.gpsimd.memset(th.ap(), cv)
        nc.const_aps.aps[(f32, cv)] = th.ap()
    nc.all_engine_barrier()
    x_d = nc.dram_tensor("x", [N_CORE, IN_F], f16, kind="ExternalInput").ap()
    w_d = nc.dram_tensor("w", [KT * 128, OUT_F], f16, kind="ExternalInput").ap()
    id_d = nc.dram_tensor("ident", [128, 128], f16, kind="ExternalInput").ap()
    y_d = nc.dram_tensor("y", [N_CORE, OUT_F], f32, kind="ExternalOutput").ap()

    with tile.TileContext(nc) as tc, ExitStack() as ctx:
        wpool = ctx.enter_context(tc.tile_pool(name="w", bufs=1))
        xpool = ctx.enter_context(tc.tile_pool(name="x", bufs=3))
        xtp = ctx.enter_context(tc.tile_pool(name="xT", bufs=5))
        kpool = ctx.enter_context(tc.tile_pool(name="ktiles", bufs=2))
        rpool = ctx.enter_context(tc.tile_pool(name="r", bufs=2))
        qpool = ctx.enter_context(tc.tile_pool(name="q", bufs=2))
        cpool = ctx.enter_context(tc.tile_pool(name="c", bufs=2))
        ypool = ctx.enter_context(tc.tile_pool(name="y", bufs=3))
        pt_pool = ctx.enter_context(tc.tile_pool(name="ptrans", bufs=2, space="PSUM"))
        po_pool = ctx.enter_context(tc.tile_pool(name="pout", bufs=6, space="PSUM"))

        w_s = wpool.tile([128, KT * OUT_F], f16, tag="w")
        ident = wpool.tile([128, 128], f16, tag="ident")
        nc.sync.dma_start(ident[:], id_d[:])

        def load_block_x(blk):
            r0 = blk * NBLK
            xts = []
            for nt in range(4):
                xt = xpool.tile([128, IN_F], f16, tag=f"x{nt}", name=f"xt{nt}")
                nc.sync.dma_start(xt[:], x_d[r0 + nt * 128: r0 + (nt + 1) * 128, :])
                xts.append(xt)
            return xts

        # block 0's x before the (long) weight DMA queue
        xts0 = load_block_x(0)
        for kt in SLOT_ORDER:
            nc.sync.dma_start(w_s[:, kt * OUT_F:(kt + 1) * OUT_F],
                              w_d[kt * 128:(kt + 1) * 128, :])

        for blk in range(NBLOCKS):
            r0 = blk * NBLK
            xcs = xts0 if blk == 0 else load_block_x(blk)
            kbs = []

            for it in range(4):
                ptr = pt_pool.tile([128, NBLK], f16, tag="ptr")
                for nt in range(4):
                    nc.tensor.transpose(ptr[:, nt * 128:(nt + 1) * 128],
                                        xcs[nt][:, it * 128:(it + 1) * 128], ident[:])
                xT = xtp.tile([128, NBLK], f16, tag="xT")
                nc.vector.tensor_copy(xT[:], ptr[:])
                kb = kpool.tile([128, NCH * NBLK], f16, tag="kbuf", name=f"kb{it}",
                                bufs=11)
                kbs.append(kb)

                # silu channel -> group 0
                nc.scalar.activation(kb[:, 0:NBLK], xT[:], Act.Silu)

                cprev = None
                for c in range(10):
                    R = rpool.tile([128, NBLK], f32, tag=f"R{c % 2}")
                    nc.vector.tensor_scalar(R[:], xT[:], BETA[c], 0.0,
                                            Alu.add, Alu.max)
                    Q = qpool.tile([128, NBLK], f32, tag=f"Q{c % 2}")
                    nc.scalar.activation(Q[:], xT[:], Act.Square, bias=BETA[c])
                    C = cpool.tile([128, NBLK], f32, tag=f"C{c % 2}")
                    nc.gpsimd.tensor_mul(C[:], Q[:], R[:])
                    if c >= 1:
                        eng = nc.vector if (c % 2) else nc.gpsimd
                        eng.tensor_sub(kb[:, c * NBLK:(c + 1) * NBLK],
                                       cprev[:], C[:])
                    cprev = C

            # GEMM: K-outer in production order, 4 row-subtiles per K-slot
            pos = [po_pool.tile([128, OUT_F], f32, tag="po", name=f"po{nsub}")
                   for nsub in range(4)]
            j = 0
            for it in range(4):
                for g in range(NCH):
                    kt = g * 4 + it
                    for nsub in range(4):
                        nc.tensor.matmul(
                            pos[nsub][:],
                            kbs[it][:, g * NBLK + nsub * 128: g * NBLK + (nsub + 1) * 128],
                            w_s[:, kt * OUT_F:(kt + 1) * OUT_F],
                            start=(j == 0), stop=(j == KT - 1))
                    j += 1
            for nsub in range(4):
                yo = ypool.tile([128, OUT_F], f32, tag="yo")
                if nsub % 2:
                    nc.vector.tensor_copy(yo[:], pos[nsub][:])
                else:
                    nc.scalar.copy(yo[:], pos[nsub][:])
                nc.sync.dma_start(y_d[r0 + nsub * 128: r0 + (nsub + 1) * 128, :], yo[:])

    nc.compile()
    _cache['nc'] = nc
    return nc


def _prep_w(base_weight, spline_weight, spline_scaler):
    swsc = (spline_weight.astype(np.float64)
            * spline_scaler.astype(np.float64)[..., None])   # [o, i, 8]
    C4 = np.array([1., -4., 6., -4., 1.])
    A = np.zeros((OUT_F, IN_F, 12), np.float64)
    for k in range(8):
        for j in range(5):
            A[:, :, k + j] += swsc[:, :, k] * C4[j]
    A /= 6.0 * 1.2 ** 3
    wD = np.cumsum(A, axis=2)[:, :, :10]                     # [o, i, 10]
    w = np.zeros((KT * 128, OUT_F), np.float64)
    for it in range(4):
        isl = slice(it * 128, (it + 1) * 128)
        w[it * 128:(it + 1) * 128, :] = base_weight.T[isl, :]       # silu slots 0..3
        for c in range(9):
            s = (c + 1) * 4 + it
            w[s * 128:(s + 1) * 128, :] = wD[:, isl, c].T
    return w.astype(np.float16)


def kernel(x, base_weight, spline_weight, spline_scaler, grid):
    x = np.asarray(x, dtype=np.float16)
    w = _prep_w(np.asarray(base_weight, np.float32),
                np.asarray(spline_weight, np.float32),
                np.asarray(spline_scaler, np.float32))
    ident = np.eye(128, dtype=np.float16)
    nc = _build()
    in_maps = []
    for c in range(NCORES):
        in_maps.append({"x": np.ascontiguousarray(x[c * N_CORE:(c + 1) * N_CORE]),
                        "w": w, "ident": ident})
    res = run_bass_kernel_spmd(nc, in_maps, core_ids=list(range(NCORES)))
    out = np.concatenate([res.results[c]["y"] for c in range(NCORES)], axis=0)
    return out.astype(np.float32)


# revision 4
# speedup vs baseline: 23602.0575x; 1.0535x over previous
"""KANLinear (B-spline) Trainium2 kernel.

Math: out = silu(x) @ Wb^T + einsum('nik,oik->no', Bspline(x), Ws*scaler)
with a uniform grid (x-knots at 1.2*c - 6.6, c = 0..11), order-3 splines.

Key identity (cardinal B-spline as truncated powers):
  b_k(t) = B(t - k),  B(s) = (1/6) sum_{j=0..4} (-1)^j C(4,j) (s-j)_+^3
With r_c = relu(x + 6.6 - 1.2c) and C_c = r_c^3, define D_c = C_c - C_{c+1}.
Then spline_out = sum_{i,c} D_c[n,i] * wD[o,i,c], where wD is a host-side
cumsum transform of spline_weight*scaler (float64). All Cox-de Boor mask /
piecewise logic folds into the GEMM weights; on-chip work per input tile is
just relu/square/multiply/subtract chains.

Numerics: channels and weights are fp16 (bf16 fails: the truncated-power
cancellation amplifies quantization ~50x; fp16 measured rel err 1.46e-3).
x is shipped as fp16 (a smooth input perturbation, ~1e-4 effect). The last
D-channel (c=9, active only for x>4.2, ~1e-5 of elements) is dropped;
measured impact +6e-6. GEMM K = (1 silu + 9 D-channels) * 512 = 5120.

Sharding: data-parallel over N across 8 cores; weights replicated.
Schedule: x is transposed by XBAR DMA-transpose straight from DRAM (no PE
transposes, no natural-layout x in SBUF); the K-outer GEMM consumes K-slots
in exactly the order the channel pipeline produces them, so the PE streams
behind the producers with no per-block barrier; a per-i-tile kbuf ring (11
tiles) keeps producers ~2.5 blocks ahead; block-0 transposes are queued
ahead of the weight DMAs. CoreSim: 286 us/core, PE 100% busy in steady
state (1280 matmuls, fp16, N=512) vs 1276 us for the mask-based baseline.
"""
import sys, os
sys.path.insert(0, '/opt/trn_rl_repo')
import numpy as np
from contextlib import ExitStack

import concourse.bass as bass
import concourse.bacc as bacc
import concourse.tile as tile
import concourse.mybir as mybir
from concourse.bass_utils import run_bass_kernel_spmd

f32 = mybir.dt.float32
f16 = mybir.dt.float16
Alu = mybir.AluOpType
Act = mybir.ActivationFunctionType

N_TOTAL, IN_F, OUT_F = 32768, 512, 512
NCORES = 8
N_CORE = N_TOTAL // NCORES          # 4096
NBLK = 512                          # rows per block
NBLOCKS = N_CORE // NBLK            # 8
NCH = 10                            # 1 silu + 9 spline D-channels
# channel 9 (active only for x>4.2, ~1e-5 of elements) is dropped: its
# post-cumsum weight is tiny; measured rel-err impact +6e-6.
KT = NCH * 4                        # 44 K-tiles of 128
BETA = [6.6 - 1.2 * c for c in range(11)]
# K-slot consumption order == production order: it-major, group-minor
SLOT_ORDER = [g * 4 + it for it in range(4) for g in range(NCH)]

_cache = {}


def _build():
    if 'nc' in _cache:
        return _cache['nc']
    nc = bacc.Bacc("TRN2", target_bir_lowering=False, debug=False, num_devices=NCORES)
    for cv in BETA[:10]:
        th = nc.alloc_sbuf_tensor(f"constk-{cv}", [128, 1], f32)
        nc.gpsimd.memset(th.ap(), cv)
        nc.const_aps.aps[(f32, cv)] = th.ap()
    nc.all_engine_barrier()
    x_d = nc.dram_tensor("x", [N_CORE, IN_F], f16, kind="ExternalInput").ap()
    w_d = nc.dram_tensor("w", [KT * 128, OUT_F], f16, kind="ExternalInput").ap()
    y_d = nc.dram_tensor("y", [N_CORE, OUT_F], f32, kind="ExternalOutput").ap()

    with tile.TileContext(nc) as tc, ExitStack() as ctx:
        wpool = ctx.enter_context(tc.tile_pool(name="w", bufs=1))
        xtp = ctx.enter_context(tc.tile_pool(name="xT", bufs=5))
        kpool = ctx.enter_context(tc.tile_pool(name="ktiles", bufs=2))
        rpool = ctx.enter_context(tc.tile_pool(name="r", bufs=2))
        qpool = ctx.enter_context(tc.tile_pool(name="q", bufs=2))
        cpool = ctx.enter_context(tc.tile_pool(name="c", bufs=2))
        ypool = ctx.enter_context(tc.tile_pool(name="y", bufs=3))
        po_pool = ctx.enter_context(tc.tile_pool(name="pout", bufs=6, space="PSUM"))

        w_s = wpool.tile([128, KT * OUT_F], f16, tag="w")

        def make_xT(blk, it):
            # XBAR DMA transpose straight from DRAM: x[rows, i-chunk] -> xT
            r0 = blk * NBLK
            xT = xtp.tile([128, NBLK], f16, tag="xT", name=f"xT{it}")
            for nt in range(4):
                nc.sync.dma_start_transpose(
                    xT[:, nt * 128:(nt + 1) * 128],
                    x_d[r0 + nt * 128: r0 + (nt + 1) * 128,
                        it * 128:(it + 1) * 128])
            return xT

        # block 0's transposes go ahead of the (long) weight DMA queue
        xT0 = [make_xT(0, it) for it in range(4)]
        for kt in SLOT_ORDER:
            nc.sync.dma_start(w_s[:, kt * OUT_F:(kt + 1) * OUT_F],
                              w_d[kt * 128:(kt + 1) * 128, :])

        for blk in range(NBLOCKS):
            r0 = blk * NBLK
            kbs = []

            for it in range(4):
                xT = xT0[it] if blk == 0 else make_xT(blk, it)
                kb = kpool.tile([128, NCH * NBLK], f16, tag="kbuf", name=f"kb{it}",
                                bufs=11)
                kbs.append(kb)

                # silu channel -> group 0
                nc.scalar.activation(kb[:, 0:NBLK], xT[:], Act.Silu)

                cprev = None
                for c in range(10):
                    R = rpool.tile([128, NBLK], f32, tag=f"R{c % 2}")
                    nc.vector.tensor_scalar(R[:], xT[:], BETA[c], 0.0,
                                            Alu.add, Alu.max)
                    Q = qpool.tile([128, NBLK], f32, tag=f"Q{c % 2}")
                    nc.scalar.activation(Q[:], xT[:], Act.Square, bias=BETA[c])
                    C = cpool.tile([128, NBLK], f32, tag=f"C{c % 2}")
                    nc.gpsimd.tensor_mul(C[:], Q[:], R[:])
                    if c >= 1:
                        eng = nc.vector if (c % 2) else nc.gpsimd
                        eng.tensor_sub(kb[:, c * NBLK:(c + 1) * NBLK],
                                       cprev[:], C[:])
                    cprev = C

            # GEMM: K-outer in production order, 4 row-subtiles per K-slot
            pos = [po_pool.tile([128, OUT_F], f32, tag="po", name=f"po{nsub}")
                   for nsub in range(4)]
            j = 0
            for it in range(4):
                for g in range(NCH):
                    kt = g * 4 + it
                    for nsub in range(4):
                        nc.tensor.matmul(
                            pos[nsub][:],
                            kbs[it][:, g * NBLK + nsub * 128: g * NBLK + (nsub + 1) * 128],
                            w_s[:, kt * OUT_F:(kt + 1) * OUT_F],
                            start=(j == 0), stop=(j == KT - 1))
                    j += 1
            for nsub in range(4):
                yo = ypool.tile([128, OUT_F], f32, tag="yo")
                if nsub % 2:
                    nc.vector.tensor_copy(yo[:], pos[nsub][:])
                else:
                    nc.scalar.copy(yo[:], pos[nsub][:])
                nc.sync.dma_start(y_d[r0 + nsub * 128: r0 + (nsub + 1) * 128, :], yo[:])

    nc.compile()
    _cache['nc'] = nc
    return nc


def _prep_w(base_weight, spline_weight, spline_scaler):
    swsc = (spline_weight.astype(np.float64)
            * spline_scaler.astype(np.float64)[..., None])   # [o, i, 8]
    C4 = np.array([1., -4., 6., -4., 1.])
    A = np.zeros((OUT_F, IN_F, 12), np.float64)
    for k in range(8):
        for j in range(5):
            A[:, :, k + j] += swsc[:, :, k] * C4[j]
    A /= 6.0 * 1.2 ** 3
    wD = np.cumsum(A, axis=2)[:, :, :10]                     # [o, i, 10]
    w = np.zeros((KT * 128, OUT_F), np.float64)
    for it in range(4):
        isl = slice(it * 128, (it + 1) * 128)
        w[it * 128:(it + 1) * 128, :] = base_weight.T[isl, :]       # silu slots 0..3
        for c in range(9):
            s = (c + 1) * 4 + it
            w[s * 128:(s + 1) * 128, :] = wD[:, isl, c].T
    return w.astype(np.float16)


def kernel(x, base_weight, spline_weight, spline_scaler, grid):
    x = np.asarray(x, dtype=np.float16)
    w = _prep_w(np.asarray(base_weight, np.float32),
                np.asarray(spline_weight, np.float32),
                np.asarray(spline_scaler, np.float32))
    nc = _build()
    in_maps = []
    for c in range(NCORES):
        in_maps.append({"x": np.ascontiguousarray(x[c * N_CORE:(c + 1) * N_CORE]),
                        "w": w})
    res = run_bass_kernel_spmd(nc, in_maps, core_ids=list(range(NCORES)))
    out = np.concatenate([res.results[c]["y"] for c in range(NCORES)], axis=0)
    return out.astype(np.float32)
